# revision 7
# baseline (speedup 1.0000x reference)
"""Trainium2 Bass kernel for nn_Aggregation (sparse block-diagonal attention).

Computation (see reference): a single learned query vector attends, per
(sentence, batch), over that sentence's 32 entity slots:
    k/v = entities @ {Wk,Wv}.T + b;  scores = q . k;  attn = softmax_e(scores)
    ctx = sum_e attn * v;            out = ctx @ Wo.T + bo

Because the query is one shared vector, the K projection folds into a tiny
fused weight computed on host:  scores[t, h] = X[t, :] @ M[:, h] + c[h] with
M = sum_d Q_bd[d, :] * Wk[d, :],  killing half of the heavy GEMM work.  The
V projection (entities @ Wv.T, ~137 GFLOP) dominates and runs as fp16
matmuls on the PE.

Sharding: data-parallel over batch, 8 of 64 batch columns per NeuronCore.
Token order inside a core is (sent, batch, entity) so the entity reduction
is contiguous in the free dimension.

Self-contained: hardcodes all shapes from the problem spec.
"""

import numpy as np
import ml_dtypes

import concourse.bass as bass
import concourse.tile as tile
from concourse import bacc, mybir, bass_utils
from concourse.masks import make_identity

# Problem constants (from spec / setup_inputs)
D = 1024
H = 16
HD = D // H
N_SENTS = 32
N_ENTS = 32
SE = N_SENTS * N_ENTS
B = 64
N_CORES = 8
BC = B // N_CORES            # batch columns per core
TOK = N_SENTS * N_ENTS * BC  # tokens per core = 8192
ST_TOK = 512                 # tokens per super-tile (2 sents x 8 b x 32 e)
N_ST = TOK // ST_TOK         # 16 super-tiles
SB = N_SENTS * BC            # (s, b) rows per core = 256

F32 = mybir.dt.float32
F16 = mybir.dt.float16

_NC_CACHE = {}


def _build():
    if "nc" in _NC_CACHE:
        return _NC_CACHE["nc"]
    nc = bacc.Bacc("TRN2", target_bir_lowering=False, debug=False)

    E = nc.dram_tensor("E", [SE, BC, D], F32, kind="ExternalInput").ap()
    WVT = nc.dram_tensor("WVT", [128, 8 * D], F16, kind="ExternalInput").ap()
    WOT = nc.dram_tensor("WOT", [128, 8 * D], F16, kind="ExternalInput").ap()
    MW = nc.dram_tensor("MW", [128, 8 * H], F16, kind="ExternalInput").ap()
    CH = nc.dram_tensor("CH", [H, 1], F32, kind="ExternalInput").ap()
    BV = nc.dram_tensor("BV", [128, 8], F32, kind="ExternalInput").ap()
    BO = nc.dram_tensor("BO", [1, D], F16, kind="ExternalInput").ap()
    MASKV = nc.dram_tensor("MASKV", [1, TOK], F16, kind="ExternalInput").ap()
    RM = nc.dram_tensor("RM", [H, 8 * 128], F16, kind="ExternalInput").ap()
    OUT = nc.dram_tensor("OUT", [SB, D], F32, kind="ExternalOutput").ap()

    with tile.TileContext(nc) as tc:
        with (
            tc.tile_pool(name="wpool", bufs=1) as wpool,
            tc.tile_pool(name="xpool", bufs=3) as xpool,
            tc.tile_pool(name="attnpool", bufs=3) as apool,
            tc.tile_pool(name="ctxpool", bufs=1) as cpool,
            tc.tile_pool(name="psT", bufs=2, space="PSUM") as psT,
            tc.tile_pool(name="psS", bufs=2, space="PSUM") as psS,
            tc.tile_pool(name="psV", bufs=2, space="PSUM") as psV,
        ):
            # ---- constants / weights (loaded once) ----
            wvt = wpool.tile([128, 8 * D], F16)
            nc.sync.dma_start(wvt[:], WVT[:])
            wot = wpool.tile([128, 8 * D], F16)
            nc.sync.dma_start(wot[:], WOT[:])
            mw = wpool.tile([128, 8 * H], F16)
            nc.sync.dma_start(mw[:], MW[:])
            ch = wpool.tile([H, 1], F32)
            nc.sync.dma_start(ch[:], CH[:])
            bv = wpool.tile([128, 8], F32)
            nc.sync.dma_start(bv[:], BV[:])
            bo = wpool.tile([1, D], F16)
            nc.sync.dma_start(bo[:], BO[:])
            maskv = wpool.tile([1, TOK], F16)
            nc.sync.dma_start(maskv[:], MASKV[:])
            rm = wpool.tile([H, 8 * 128], F16)
            nc.sync.dma_start(rm[:], RM[:])
            ident = wpool.tile([128, 128], F16)
            make_identity(nc, ident)
            ones = wpool.tile([1, 128], F16)
            nc.vector.memset(ones[:], 1.0)

            # ctx^T accumulator: [d-in-chunk, chunk * SB + sb] fp32
            ctxT = cpool.tile([128, 8 * SB], F32)

            for st in range(N_ST):
                s0 = 2 * st
                # ---- load X (4 subtiles of [128, D] fp32), cast to bf16 ----
                xf = xpool.tile([128, 4 * D], F32, tag="xf")
                for j in range(4):
                    s = s0 + j // 2
                    b0 = 4 * (j % 2)
                    src = E[s * N_ENTS:(s + 1) * N_ENTS, b0:b0 + 4, :]
                    src = src.transpose([1, 0, 2])  # (b, e, d) row order
                    nc.sync.dma_start(xf[:, j * D:(j + 1) * D], src)
                xbf = xpool.tile([128, 4 * D], F16, tag="xbf")
                for j in range(4):
                    nc.scalar.copy(xbf[:, j * D:(j + 1) * D],
                                   xf[:, j * D:(j + 1) * D])

                # ---- transpose to X^T [c-in-chunk, chunk * 512 + t] bf16 ----
                xt = xpool.tile([128, 8 * ST_TOK], F16, tag="xt")
                for c in range(8):
                    pt = psT.tile([128, ST_TOK], F16, tag="pt")
                    for j in range(4):
                        nc.tensor.transpose(
                            pt[:, j * 128:(j + 1) * 128],
                            xbf[:, j * D + c * 128: j * D + (c + 1) * 128],
                            ident[:],
                        )
                    nc.scalar.copy(xt[:, c * ST_TOK:(c + 1) * ST_TOK], pt[:])

                # ---- scores^T [16 h, 512 t] = M^T X^T + mask ----
                ps_s = psS.tile([H, ST_TOK], F32, tag="ps_s")
                for c in range(8):
                    nc.tensor.matmul(
                        ps_s[:],
                        mw[:, c * H:(c + 1) * H],
                        xt[:, c * ST_TOK:(c + 1) * ST_TOK],
                        start=(c == 0), stop=False,
                    )
                nc.tensor.matmul(
                    ps_s[:], ones[:, :H],
                    maskv[:, st * ST_TOK:(st + 1) * ST_TOK],
                    start=False, stop=True,
                )

                # ---- softmax over e (32, contiguous) ----
                attn = apool.tile([H, ST_TOK], F16, tag="attn")
                nc.scalar.activation(attn[:], ps_s[:],
                                     mybir.ActivationFunctionType.Exp,
                                     bias=ch[:])
                zsum = apool.tile([H, 16], F32, tag="zsum")
                nc.vector.reduce_sum(
                    zsum[:], attn[:].rearrange("p (g e) -> p g e", e=N_ENTS),
                    axis=mybir.AxisListType.X)
                zrec = apool.tile([H, 16], F32, tag="zrec")
                nc.vector.reciprocal(zrec[:], zsum[:])
                attn_n = apool.tile([H, ST_TOK], F16, tag="attn_n")
                nc.vector.tensor_mul(
                    attn_n[:].rearrange("p (g e) -> p g e", e=N_ENTS),
                    attn[:].rearrange("p (g e) -> p g e", e=N_ENTS),
                    zrec[:].broadcast_to((H, 16, N_ENTS)),
                )

                # ---- V projection + weighted e-reduction, per dout chunk ----
                for m in range(8):
                    # expand attn rows (2m, 2m+1) to 128 partitions:
                    # psum_ae = RM_m.T @ attn_n  (RM_m[h, p] = [h == 2m + p//64])
                    ps_a = psS.tile([128, ST_TOK], F32, tag="ps_a")
                    nc.tensor.matmul(
                        ps_a[:], rm[:, m * 128:(m + 1) * 128], attn_n[:],
                        start=True, stop=True,
                    )
                    ax = apool.tile([128, ST_TOK], F16, tag="ax")
                    nc.scalar.copy(ax[:], ps_a[:])
                    ps_v = psV.tile([128, ST_TOK], F32, tag="ps_v")
                    for c in range(8):
                        nc.tensor.matmul(
                            ps_v[:],
                            wvt[:, c * D + m * 128: c * D + (m + 1) * 128],
                            xt[:, c * ST_TOK:(c + 1) * ST_TOK],
                            start=(c == 0), stop=(c == 7),
                        )
                    prod = apool.tile([128, ST_TOK], F16, tag="prod")
                    nc.vector.tensor_mul(prod[:], ps_v[:], ax[:])
                    nc.vector.reduce_sum(
                        ctxT[:, m * SB + st * 16: m * SB + (st + 1) * 16],
                        prod[:].rearrange("p (g e) -> p g e", e=N_ENTS),
                        axis=mybir.AxisListType.X)

            # ---- bias bv (attn sums to 1 -> ctx += bv), cast to bf16 ----
            ctxT_bf = cpool.tile([128, 8 * SB], F16)
            for m in range(8):
                nc.vector.tensor_add(
                    ctxT_bf[:, m * SB:(m + 1) * SB],
                    ctxT[:, m * SB:(m + 1) * SB],
                    bv[:, m:m + 1].broadcast_to((128, SB)),
                )

            # ---- out projection: OUT[sb, f] = ctx^T.T @ WoT + bo ----
            for mt in range(2):
                fin = cpool.tile([128, D], F32, tag="fin")
                for nh in range(2):
                    ps_f = psV.tile([128, 512], F32, tag="ps_v")
                    for c in range(8):
                        nc.tensor.matmul(
                            ps_f[:],
                            ctxT_bf[:, c * SB + mt * 128: c * SB + (mt + 1) * 128],
                            wot[:, c * D + nh * 512: c * D + (nh + 1) * 512],
                            start=(c == 0), stop=False,
                        )
                    nc.tensor.matmul(
                        ps_f[:], ones[:, :128],
                        bo[:, nh * 512:(nh + 1) * 512],
                        start=False, stop=True,
                    )
                    nc.scalar.copy(fin[:, nh * 512:(nh + 1) * 512], ps_f[:])
                nc.sync.dma_start(OUT[mt * 128:(mt + 1) * 128, :], fin[:])

    nc.compile()
    _NC_CACHE["nc"] = nc
    return nc


def _prep_host(entities, padding_mask, n_sents, query, in_proj_w, in_proj_b,
               out_proj_w, out_proj_b):
    """Host-side prep: shard + weight fusion/packing. Returns in_maps."""
    assert int(n_sents) == N_SENTS
    bf = np.float16
    f32 = np.float32

    Wq = in_proj_w[:D]
    Wk = in_proj_w[D:2 * D]
    Wv = in_proj_w[2 * D:]
    bq = in_proj_b[:D]
    bk = in_proj_b[D:2 * D]
    bv = in_proj_b[2 * D:]
    scale = f32(1.0 / np.sqrt(HD))

    q_vec = ((query.astype(np.float64) @ Wq.T.astype(np.float64)
              + bq.astype(np.float64)) * scale)
    # M[c, h] = sum_hd q_vec[h*HD+hd] * Wk[h*HD+hd, c];  c_h = q_vec_h . bk_h
    M = np.stack(
        [q_vec[h * HD:(h + 1) * HD] @ Wk.astype(np.float64)[h * HD:(h + 1) * HD, :]
         for h in range(H)], axis=1)  # [D, H]
    c_h = np.array(
        [q_vec[h * HD:(h + 1) * HD] @ bk.astype(np.float64)[h * HD:(h + 1) * HD]
         for h in range(H)])

    def pack_kxn(w_t):  # [1024, N] -> [128, 8*N] chunk-major
        n = w_t.shape[1]
        return np.ascontiguousarray(
            w_t.reshape(8, 128, n).transpose(1, 0, 2).reshape(128, 8 * n))

    WVT = pack_kxn(Wv.T.astype(f32)).astype(bf)
    WOT = pack_kxn(out_proj_w.T.astype(f32)).astype(bf)
    MW = pack_kxn(M.astype(f32)).astype(bf)
    CH = c_h.astype(f32).reshape(H, 1)
    BVp = np.ascontiguousarray(bv.astype(f32).reshape(8, 128).T)  # [128, 8]
    BOp = out_proj_b.astype(f32).reshape(1, D).astype(bf)
    RMp = np.zeros((H, 8 * 128), dtype=bf)
    for m in range(8):
        for p in range(128):
            RMp[2 * m + p // 64, m * 128 + p] = 1.0

    in_maps = []
    for core in range(N_CORES):
        bsl = slice(core * BC, (core + 1) * BC)
        e_c = np.ascontiguousarray(entities[:, bsl, :], dtype=f32)
        # mask values in (s, b, e) token order
        m_c = padding_mask[:, bsl].reshape(N_SENTS, N_ENTS, BC)
        m_c = np.ascontiguousarray(m_c.transpose(0, 2, 1)).reshape(1, TOK)
        maskv = (m_c.astype(f32) * f32(-30000.0)).astype(bf)
        in_maps.append({
            "E": e_c, "WVT": WVT, "WOT": WOT, "MW": MW, "CH": CH,
            "BV": BVp, "BO": BOp, "MASKV": maskv, "RM": RMp,
        })
    return in_maps


def kernel(entities, padding_mask, n_sents, query, in_proj_w, in_proj_b,
           out_proj_w, out_proj_b):
    in_maps = _prep_host(entities, padding_mask, n_sents, query, in_proj_w,
                         in_proj_b, out_proj_w, out_proj_b)
    nc = _build()
    res = bass_utils.run_bass_kernel_spmd(
        nc, in_maps=in_maps, core_ids=list(range(N_CORES)))
    out = np.empty((N_SENTS, B, D), dtype=np.float32)
    for core in range(N_CORES):
        o = res.results[core]["OUT"].reshape(N_SENTS, BC, D)
        out[:, core * BC:(core + 1) * BC, :] = o
    return out


# revision 12
# speedup vs baseline: 2.5973x; 2.5973x over previous
"""Trainium2 Bass kernel for nn_Aggregation (sparse block-diagonal attention).

Computation (see reference): a single learned query vector attends, per
(sentence, batch), over that sentence's 32 entity slots:
    k/v = entities @ {Wk,Wv}.T + b;  scores = q . k;  attn = softmax_e(scores)
    ctx = sum_e attn * v;            out = ctx @ Wo.T + bo

Two algebraic reductions make this cheap:
 1. The query is one shared vector, so the K projection folds into a tiny
    fused weight computed on host: scores[t, h] = X[t, :] @ M[:, h] + c_h
    with M[c, h] = sum_hd q[h, hd] * Wk[h*64+hd, c].  The K GEMM vanishes.
 2. The attention weights do not depend on the feature dim, so the
    entity-average commutes with the (linear) V projection:
       ctx[(s,b), d] = sum_c Wv[d, c] * Y[h(d), c, (s,b)],
       Y[h, c, (s,b)] = sum_e attn[s,b,h,e] * X[(s,e,b), c].
    Y costs ~0.5 GMAC/core instead of the 8.6 GMAC V projection.

All matmuls run in fp16 (10 mantissa bits; measured end-to-end error vs
the fp32 reference is ~3e-3 of output RMS, ~10x better than bf16).

Sharding: data-parallel over batch, 8 of 64 batch columns per core.
Token order is the natural (sent, entity, batch); the host pre-packs the
activation shard in fp16 twice: natural X (tokens on partitions) and
transposed X^T (contraction dim on partitions), so the device spends its
cycles on matmuls only.

Per 512-token super-tile (2 sents, 16 (s,b) groups):
  scores^T[h, t'] = M^T @ X^T (+ mask via K=1 matmul)        [PE]
  attn = exp(scores + c_h); attn_n = attn / sum_e            [ACT + DVE]
  attn_exp2[t', (sb,h)] = (R^T @ attn_n) * blockmask         [PE + DVE]
  Y^T[c, (sb, h)] += X_nat.T @ attn_exp2  (over 4 subtiles)  [PE]
Epilogue:
  ctx^T[d, sb] = sum_c Wv^T[c, d] Y^T[c, (sb, h(d))]  (per-head matmuls,
  two heads col-packed per PSUM tile) ; += bv; out = ctx^T.T @ Wo^T + bo.

Self-contained: hardcodes all shapes from the problem spec.
"""

import numpy as np

import concourse.bass as bass
import concourse.tile as tile
from concourse import bacc, mybir, bass_utils

# Problem constants (from spec / setup_inputs)
D = 1024
H = 16
HD = D // H
N_SENTS = 32
N_ENTS = 32
SE = N_SENTS * N_ENTS
B = 64
N_CORES = 8
BC = B // N_CORES            # batch columns per core
TOK = N_SENTS * N_ENTS * BC  # tokens per core = 8192
ST_TOK = 512                 # tokens per super-tile (2 sents x 32 e x 8 b)
N_ST = TOK // ST_TOK         # 16 super-tiles
SB = N_SENTS * BC            # (s, b) rows per core = 256

F32 = mybir.dt.float32
F16 = mybir.dt.float16

_NC_CACHE = {}


def _build():
    if "nc" in _NC_CACHE:
        return _NC_CACHE["nc"]
    nc = bacc.Bacc("TRN2", target_bir_lowering=False, debug=False)

    # X^T: [c-in-chunk(128), chunk(8) * TOK + t'], t' = (s, e, b) natural
    XT = nc.dram_tensor("XT", [128, 8 * TOK], F16, kind="ExternalInput").ap()
    # X natural: [t', c]
    XN = nc.dram_tensor("XN", [TOK, D], F16, kind="ExternalInput").ap()
    WVT = nc.dram_tensor("WVT", [128, 8 * D], F16, kind="ExternalInput").ap()
    WOT = nc.dram_tensor("WOT", [128, 8 * D], F16, kind="ExternalInput").ap()
    MW = nc.dram_tensor("MW", [128, 8 * H], F16, kind="ExternalInput").ap()
    CH = nc.dram_tensor("CH", [H, 1], F32, kind="ExternalInput").ap()
    BV = nc.dram_tensor("BV", [128, 8], F32, kind="ExternalInput").ap()
    BO = nc.dram_tensor("BO", [1, D], F16, kind="ExternalInput").ap()
    MASKV = nc.dram_tensor("MASKV", [1, TOK], F16, kind="ExternalInput").ap()
    # R4: [h, combo] = [h == hg(combo)*8 + h_lo(combo)],
    #     combo = hg*128 + sbl*8 + h_lo
    R4 = nc.dram_tensor("R4", [H, 256], F16, kind="ExternalInput").ap()
    # block masks per sent-parity: [row r, combo] = [sbl(combo) == jp*8 + r%8]
    BM = nc.dram_tensor("BM", [128, 2 * 256], F16, kind="ExternalInput").ap()
    OUT = nc.dram_tensor("OUT", [SB, D], F32, kind="ExternalOutput").ap()

    with tile.TileContext(nc) as tc:
        with (
            tc.tile_pool(name="wpool", bufs=1) as wpool,
            tc.tile_pool(name="xpool", bufs=3) as xpool,
            tc.tile_pool(name="attnpool", bufs=3) as apool,
            tc.tile_pool(name="ctxpool", bufs=1) as cpool,
            tc.tile_pool(name="psS", bufs=2, space="PSUM") as psS,
            tc.tile_pool(name="psQ", bufs=2, space="PSUM") as psQ,
            tc.tile_pool(name="psY", bufs=2, space="PSUM") as psY,
            tc.tile_pool(name="psC", bufs=2, space="PSUM") as psC,
        ):
            # ---- constants / weights (loaded once) ----
            wvt = wpool.tile([128, 8 * D], F16)
            nc.sync.dma_start(wvt[:], WVT[:])
            wot = wpool.tile([128, 8 * D], F16)
            nc.sync.dma_start(wot[:], WOT[:])
            mw = wpool.tile([128, 8 * H], F16)
            nc.sync.dma_start(mw[:], MW[:])
            ch = wpool.tile([H, 1], F32)
            nc.sync.dma_start(ch[:], CH[:])
            bv = wpool.tile([128, 8], F32)
            nc.sync.dma_start(bv[:], BV[:])
            bo = wpool.tile([1, D], F16)
            nc.sync.dma_start(bo[:], BO[:])
            maskv = wpool.tile([1, TOK], F16)
            nc.sync.dma_start(maskv[:], MASKV[:])
            r4 = wpool.tile([H, 256], F16)
            nc.sync.dma_start(r4[:], R4[:])
            bm = wpool.tile([128, 2 * 256], F16)
            nc.sync.dma_start(bm[:], BM[:])
            ones = wpool.tile([1, 128], F16)
            nc.vector.memset(ones[:], 1.0)

            # Y^T accumulator: [c-in-chunk, chunk(8) * (H * SB) + h * SB + sb]
            yt = cpool.tile([128, 8 * H * SB], F16)

            for st in range(N_ST):
                t0 = st * ST_TOK
                # ---- load X^T and X natural for this super-tile ----
                xt = xpool.tile([128, 8 * ST_TOK], F16, tag="xt")
                for c in range(8):
                    nc.sync.dma_start(
                        xt[:, c * ST_TOK:(c + 1) * ST_TOK],
                        XT[:, c * TOK + t0: c * TOK + t0 + ST_TOK])
                xn = xpool.tile([128, 4 * D], F16, tag="xn")
                for j in range(4):
                    nc.sync.dma_start(
                        xn[:, j * D:(j + 1) * D],
                        XN[t0 + j * 128: t0 + (j + 1) * 128, :])

                # ---- scores^T [16 h, 512 t'] = M^T X^T + mask ----
                ps_s = psS.tile([H, ST_TOK], F32, tag="ps_s")
                for c in range(8):
                    nc.tensor.matmul(
                        ps_s[:],
                        mw[:, c * H:(c + 1) * H],
                        xt[:, c * ST_TOK:(c + 1) * ST_TOK],
                        start=(c == 0), stop=False,
                    )
                nc.tensor.matmul(
                    ps_s[:], ones[:, :H],
                    maskv[:, t0:t0 + ST_TOK],
                    start=False, stop=True,
                )

                # ---- softmax over e (stride BC inside (s, e, b)) ----
                attn = apool.tile([H, ST_TOK], F16, tag="attn")
                nc.scalar.activation(attn[:], ps_s[:],
                                     mybir.ActivationFunctionType.Exp,
                                     bias=ch[:])
                zsum = apool.tile([H, 16], F32, tag="zsum")
                nc.vector.reduce_sum(
                    zsum[:],
                    attn[:].rearrange("p (s e b) -> p s b e", e=N_ENTS, b=BC),
                    axis=mybir.AxisListType.X)
                zrec = apool.tile([H, 16], F32, tag="zrec")
                nc.vector.reciprocal(zrec[:], zsum[:])
                attn_n = apool.tile([H, ST_TOK], F16, tag="attn_n")
                nc.vector.tensor_mul(
                    attn_n[:].rearrange("p (s e b) -> p s e b", e=N_ENTS, b=BC),
                    attn[:].rearrange("p (s e b) -> p s e b", e=N_ENTS, b=BC),
                    zrec[:].rearrange("p (s b) -> p s b", b=BC)[:, :, None, :]
                    .broadcast_to((H, 2, N_ENTS, BC)),
                )

                # ---- attn_exp2[j]: [128 t'-rows, 256 (hg, sbl, h_lo)] ----
                ax2 = apool.tile([128, 4 * 256], F16, tag="ax2")
                for j in range(4):
                    ps_q = psQ.tile([128, 256], F32, tag="ps_q")
                    nc.tensor.matmul(
                        ps_q[:], attn_n[:, j * 128:(j + 1) * 128], r4[:],
                        start=True, stop=True,
                    )
                    nc.vector.tensor_mul(
                        ax2[:, j * 256:(j + 1) * 256], ps_q[:],
                        bm[:, (j // 2) * 256:(j // 2 + 1) * 256],
                    )

                # ---- Y^T += X_nat.T @ attn_exp2, per c-slice ----
                for cs in range(8):
                    ps_y = psY.tile([128, 256], F32, tag="ps_y")
                    for j in range(4):
                        nc.tensor.matmul(
                            ps_y[:],
                            xn[:, j * D + cs * 128: j * D + (cs + 1) * 128],
                            ax2[:, j * 256:(j + 1) * 256],
                            start=(j == 0), stop=(j == 3),
                        )
                    # scatter copy into yt: psum col hg*128 + sbl*8 + h_lo
                    #  -> yt col cs*(H*SB) + (hg*8 + h_lo)*SB + st*16 + sbl
                    ytv = yt[:].rearrange("p (ch sb) -> p ch sb", sb=SB)
                    for hg in range(2):
                        src = ps_y[:, hg * 128:(hg + 1) * 128]
                        src = src.rearrange("p (sbl hl) -> p hl sbl", hl=8)
                        dst = ytv[:, cs * H + hg * 8: cs * H + hg * 8 + 8,
                                  st * 16: st * 16 + 16]
                        nc.scalar.copy(dst, src)

            # ---- ctx^T per head-pair: [128 d, 256 sb] via col-packed MMs ----
            ctxT_bf = cpool.tile([128, 8 * SB], F16)
            for m0 in range(8):
                ps_ctx = psC.tile([128, SB], F32, tag="ps_c")
                for hh in range(2):
                    h = 2 * m0 + hh
                    for c in range(8):
                        nc.tensor.matmul(
                            ps_ctx[hh * 64:(hh + 1) * 64, :],
                            wvt[:, c * D + h * HD: c * D + h * HD + HD],
                            yt[:, c * (H * SB) + h * SB: c * (H * SB) + (h + 1) * SB],
                            start=(c == 0), stop=(c == 7),
                            tile_position=(0, hh * 64),
                        )
                nc.vector.tensor_add(
                    ctxT_bf[:, m0 * SB:(m0 + 1) * SB],
                    ps_ctx[:],
                    bv[:, m0:m0 + 1].broadcast_to((128, SB)),
                )

            # ---- out projection: OUT[sb, f] = ctx^T.T @ WoT + bo ----
            for mt in range(2):
                fin = cpool.tile([128, D], F32, tag="fin")
                for nh in range(2):
                    ps_f = psC.tile([128, 512], F32, tag="ps_c")
                    for c in range(8):
                        nc.tensor.matmul(
                            ps_f[:],
                            ctxT_bf[:, c * SB + mt * 128: c * SB + (mt + 1) * 128],
                            wot[:, c * D + nh * 512: c * D + (nh + 1) * 512],
                            start=(c == 0), stop=False,
                        )
                    nc.tensor.matmul(
                        ps_f[:], ones[:, :128],
                        bo[:, nh * 512:(nh + 1) * 512],
                        start=False, stop=True,
                    )
                    nc.scalar.copy(fin[:, nh * 512:(nh + 1) * 512], ps_f[:])
                nc.sync.dma_start(OUT[mt * 128:(mt + 1) * 128, :], fin[:])

    nc.compile()
    _NC_CACHE["nc"] = nc
    return nc


def _prep_host(entities, padding_mask, n_sents, query, in_proj_w, in_proj_b,
               out_proj_w, out_proj_b):
    """Host-side prep: shard + layout/dtype packing + weight fusion."""
    assert int(n_sents) == N_SENTS
    f16 = np.float16
    f32 = np.float32

    Wq = in_proj_w[:D]
    Wk = in_proj_w[D:2 * D]
    Wv = in_proj_w[2 * D:]
    bq = in_proj_b[:D]
    bk = in_proj_b[D:2 * D]
    bv = in_proj_b[2 * D:]
    scale = np.float64(1.0) / np.sqrt(np.float64(HD))

    q_vec = ((query.astype(np.float64) @ Wq.T.astype(np.float64)
              + bq.astype(np.float64)) * scale)
    # M[c, h] = sum_hd q_vec[h*HD+hd] * Wk[h*HD+hd, c];  c_h = q_vec_h . bk_h
    M = np.stack(
        [q_vec[h * HD:(h + 1) * HD] @ Wk.astype(np.float64)[h * HD:(h + 1) * HD, :]
         for h in range(H)], axis=1)  # [D, H]
    c_h = np.array(
        [q_vec[h * HD:(h + 1) * HD] @ bk.astype(np.float64)[h * HD:(h + 1) * HD]
         for h in range(H)])

    def pack_kxn(w_t):  # [1024, N] -> [128, 8*N] chunk-major
        n = w_t.shape[1]
        return np.ascontiguousarray(
            w_t.reshape(8, 128, n).transpose(1, 0, 2).reshape(128, 8 * n))

    WVT = pack_kxn(Wv.T.astype(f32)).astype(f16)
    WOT = pack_kxn(out_proj_w.T.astype(f32)).astype(f16)
    MW = pack_kxn(M.astype(f32)).astype(f16)
    CH = c_h.astype(f32).reshape(H, 1)
    BVp = np.ascontiguousarray(bv.astype(f32).reshape(8, 128).T)  # [128, 8]
    BOp = out_proj_b.astype(f32).reshape(1, D).astype(f16)

    # R4[h, combo] = [h == hg*8 + h_lo], combo = hg*128 + sbl*8 + h_lo
    R4p = np.zeros((H, 256), dtype=f16)
    for combo in range(256):
        hg, rem = divmod(combo, 128)
        h_lo = rem % 8
        R4p[hg * 8 + h_lo, combo] = 1.0
    # BM[r, jp*256 + combo] = [sbl(combo) == jp*8 + r%8]
    BMp = np.zeros((128, 2 * 256), dtype=f16)
    for r in range(128):
        for jp in range(2):
            for combo in range(256):
                sbl = (combo % 128) // 8
                if sbl == jp * 8 + r % 8:
                    BMp[r, jp * 256 + combo] = 1.0

    ent16 = entities.astype(f16)  # [SE, B, D]
    maskf = padding_mask.astype(f32) * f32(-30000.0)

    in_maps = []
    for core in range(N_CORES):
        bsl = slice(core * BC, (core + 1) * BC)
        xn = np.ascontiguousarray(ent16[:, bsl, :].reshape(TOK, D))
        # X^T, chunk-major: [128, c * TOK + t'], t' = (s, e, b) natural order
        xt = xn.T  # [D, TOK]
        xt = np.ascontiguousarray(
            xt.reshape(8, 128, TOK).transpose(1, 0, 2).reshape(128, 8 * TOK))
        maskv = np.ascontiguousarray(
            maskf[:, bsl].reshape(1, TOK)).astype(f16)
        in_maps.append({
            "XT": xt, "XN": xn, "WVT": WVT, "WOT": WOT, "MW": MW, "CH": CH,
            "BV": BVp, "BO": BOp, "MASKV": maskv, "R4": R4p, "BM": BMp,
        })
    return in_maps


def kernel(entities, padding_mask, n_sents, query, in_proj_w, in_proj_b,
           out_proj_w, out_proj_b):
    in_maps = _prep_host(entities, padding_mask, n_sents, query, in_proj_w,
                         in_proj_b, out_proj_w, out_proj_b)
    nc = _build()
    res = None
    last_err = None
    for attempt in range(3):
        try:
            res = bass_utils.run_bass_kernel_spmd(
                nc, in_maps=in_maps, core_ids=list(range(N_CORES)))
            break
        except Exception as e:  # rare transient device wedge; retry
            last_err = e
            import time as _time
            _time.sleep(3)
    if res is None:
        raise last_err
    out = np.empty((N_SENTS, B, D), dtype=np.float32)
    for core in range(N_CORES):
        o = res.results[core]["OUT"].reshape(N_SENTS, BC, D)
        out[:, core * BC:(core + 1) * BC, :] = o
    return out
